# revision 1
# baseline (speedup 1.0000x reference)
"""Trainium2 Bass kernel for nn_Corr (stereo disparity correlation).

Math: reference computes, per (b,h,w):
    out = (1/(81*C)) * sum_c [ x*Sy + y*Sx ]
where Sx[w] = sum_{d=0..40} x[w+d]  (zero-padded beyond W)
      Sy[w] = sum_{d=1..40} y[w-d]  (zero-padded below 0)

Sharding: data-parallel over (batch, H/2) -> 8 cores, no communication.

Per-core pipeline (R = 128 (b,h) rows on this core):
  - Partition convention p = 2c + r  (c = channel, r = row-half): row pair u
    holds rows (u, u + R/2). This makes the HBM partition stride uniform, so
    each (tensor, group) loads with ONE 128-partition 3-dim DMA (~2 MiB).
  - DMA lands x in a zero-padded fp32 buffer [42|x 512|42] (stride 596),
    y in [41|y 512] (stride 553); GpSimd memsets the pads.
  - tensor_tensor_scan computes each sliding sum in one pass (fp32 in,
    bf16 out; throughput is dtype-independent):
        Sx[w] = Sx[w-1] + x[w+40] - x[w-1]
        Sy[w] = Sy[w-1] + y[w-1] - y[w-41]
    x-scans run on GpSimd, y-scans mostly on VectorE (load balance).
  - ScalarE casts x,y fp32 -> bf16 (contiguous tiles) for the products.
  - Products P1 = x*Sy, P2 = y*Sx on VectorE (bf16 2x mode).
  - TensorE reduces over channels with a constant block-ones stationary
    (partition k = 2c+r -> output row m = u + (R/2)*(k%2)), accumulating
    16 pairs per PSUM tile (4 tiles), so output drains overlap compute.
  - ScalarE copies each finished PSUM quarter -> SBUF with the 1/(81*C)
    scale; 4 output DMAs.
"""
import numpy as np

import concourse.bass as bass
import concourse.tile as tile
from concourse import bacc, mybir
from concourse.bass_utils import run_bass_kernel_spmd

N_CORES = 8
B, C, H, W = 4, 64, 256, 512
MAXD = 40
D = 2 * MAXD + 1  # 81
ROWS_PER_CORE = B * H // N_CORES  # 128
SCALE = 1.0 / (D * C)

XPAD = 42
XSTR = 596   # [42 zeros | x 512 | 42 zeros]
YPAD = 41
YSTR = 553   # [41 zeros | y 512]

F32 = mybir.dt.float32
BF16 = mybir.dt.bfloat16
AOP = mybir.AluOpType
AF = mybir.ActivationFunctionType


def make_ones_const(n_rows: int = ROWS_PER_CORE) -> np.ndarray:
    """Z[k, 63 + (n_rows//2)*(k%2)] = 1. lhsT for pair u is Z[:, 63-u : 191-u],
    mapping partition k = 2c+r to output row m = u + (n_rows//2)*r."""
    import ml_dtypes
    z = np.zeros((128, 192), dtype=ml_dtypes.bfloat16)
    half = n_rows // 2
    z[0:128:2, 63] = 1
    z[1:128:2, 63 + half] = 1
    return z


def _groups(n_pairs):
    """(start_pair, T) list: small prologue groups for fast pipeline rampup
    and small epilogue groups for a short drain tail."""
    if n_pairs <= 8:
        return [(u, 2) for u in range(0, n_pairs, 2)]
    pro = [2, 2, 4]
    epi = [4, 2, 2]
    mid = n_pairs - sum(pro) - sum(epi)
    assert mid >= 0 and mid % 8 == 0
    sizes = pro + [8] * (mid // 8) + epi
    out = []
    u = 0
    for T in sizes:
        out.append((u, T))
        u += T
    return out


def build(n_rows: int = ROWS_PER_CORE):
    assert n_rows % 2 == 0
    n_pairs = n_rows // 2
    half = n_rows // 2
    # PSUM output drains must start at 32-aligned partitions: split pairs
    # into halves of 32 when possible, else use one accumulation group.
    qsize = 32 if n_pairs % 32 == 0 else n_pairs
    n_q = n_pairs // qsize

    nc = bacc.Bacc("TRN2", target_bir_lowering=False, debug=False,
                   num_devices=N_CORES)
    xs = nc.dram_tensor("xs", [C, n_rows, W], F32, kind="ExternalInput").ap()
    ys = nc.dram_tensor("ys", [C, n_rows, W], F32, kind="ExternalInput").ap()
    zs = nc.dram_tensor("zs", [128, 192], BF16, kind="ExternalInput").ap()
    os_ = nc.dram_tensor("os", [n_rows, W], F32, kind="ExternalOutput").ap()

    # p = 2c + r <-> h = r*half + u ; HBM offset(p, u, w) linear in p
    xs_v = xs.rearrange("c (r u) w -> (c r) u w", r=2)
    ys_v = ys.rearrange("c (r u) w -> (c r) u w", r=2)

    with tile.TileContext(nc) as tc:
        with (
            tc.tile_pool(name="const", bufs=1) as constp,
            tc.tile_pool(name="xpf", bufs=3) as xpfp,
            tc.tile_pool(name="ypf", bufs=3) as ypfp,
            tc.tile_pool(name="xbf", bufs=2) as xbfp,
            tc.tile_pool(name="ybf", bufs=2) as ybfp,
            tc.tile_pool(name="sx", bufs=2) as sxp,
            tc.tile_pool(name="sy", bufs=2) as syp,
            tc.tile_pool(name="prod", bufs=8) as prodp,
            tc.tile_pool(name="outp", bufs=1) as outp,
            tc.tile_pool(name="ps", bufs=1, space="PSUM") as psp,
        ):
            z_sb = constp.tile([128, 192], BF16)
            nc.sync.dma_start(z_sb[:], zs)

            out_sb = outp.tile([128, W], F32)
            psum_ts = [psp.tile([128, W], F32, tag=f"q{q}", name=f"psum_q{q}")
                       for q in range(n_q)]

            for (u0, T) in _groups(n_pairs):
                # ---- one 128-partition DMA per tensor ----
                xpf = xpfp.tile([128, T * XSTR], F32, tag="xpf")
                ypf = ypfp.tile([128, T * YSTR], F32, tag="ypf")
                xp3 = xpf[:].rearrange("p (t q) -> p t q", q=XSTR)
                yp3 = ypf[:].rearrange("p (t q) -> p t q", q=YSTR)
                nc.scalar.memzero(xp3[:, :, 0:XPAD])
                nc.scalar.memzero(xp3[:, :, XPAD + W:XSTR])
                nc.scalar.memzero(yp3[:, :, 0:YPAD])
                nc.sync.dma_start(xp3[:, :, XPAD:XPAD + W],
                                  xs_v[:, u0:u0 + T, :])
                nc.sync.dma_start(yp3[:, :, YPAD:YSTR],
                                  ys_v[:, u0:u0 + T, :])

                # ---- casts fp32 -> bf16 (contiguous) for products ----
                xbf = xbfp.tile([128, T * W], BF16, tag="xbf")
                ybf = ybfp.tile([128, T * W], BF16, tag="ybf")
                xb3 = xbf[:].rearrange("p (t w) -> p t w", w=W)
                yb3 = ybf[:].rearrange("p (t w) -> p t w", w=W)
                nc.scalar.activation(xb3[:], xp3[:, :, XPAD:XPAD + W], AF.Copy)
                nc.scalar.activation(yb3[:], yp3[:, :, YPAD:YSTR], AF.Copy)

                sxt = sxp.tile([128, T * 553], BF16, tag="sx")
                syt = syp.tile([128, T * W], BF16, tag="sy")
                for t in range(T):
                    u = u0 + t
                    qx = t * XSTR
                    qy = t * YSTR
                    # Sx scan (VectorE; walrus rejects scans on GpSimd):
                    # out[i] = Sx[i-41], i in [0, 553)
                    nc.vector.tensor_tensor_scan(
                        sxt[:, t * 553:(t + 1) * 553],
                        xpf[:, qx + 41: qx + 594],
                        xpf[:, qx: qx + 553],
                        0.0, op0=AOP.add, op1=AOP.subtract)
                    # Sy scan: out[i] = Sy[i]
                    nc.vector.tensor_tensor_scan(
                        syt[:, t * W:(t + 1) * W],
                        ypf[:, qy + YPAD - 1: qy + YPAD - 1 + W],
                        ypf[:, qy: qy + W],
                        0.0, op0=AOP.add, op1=AOP.subtract)

                    p1 = prodp.tile([128, W], BF16, tag="p1")
                    p2 = prodp.tile([128, W], BF16, tag="p2")
                    # products: ~2/3 of P1 on VectorE (2x bf16), rest +
                    # all P2 on GpSimd, balancing DVE scan load vs Pool
                    p1_eng = nc.vector if u % 3 != 2 else nc.gpsimd
                    p1_eng.tensor_tensor(
                        p1[:], xbf[:, t * W:(t + 1) * W],
                        syt[:, t * W:(t + 1) * W], AOP.mult)
                    nc.gpsimd.tensor_tensor(
                        p2[:], ybf[:, t * W:(t + 1) * W],
                        sxt[:, t * 553 + 41: t * 553 + 553], AOP.mult)
                    p1 = p1[:]
                    p2 = p2[:]

                    q = u // qsize
                    lhs = z_sb[:, 63 - u: 191 - u]
                    nc.tensor.matmul(psum_ts[q][:], lhs, p1,
                                     start=(u % qsize == 0), stop=False)
                    nc.tensor.matmul(psum_ts[q][:], lhs, p2,
                                     start=False, stop=(u % qsize == qsize - 1))

                    if u % qsize == qsize - 1:
                        # accumulation group q complete: scale-copy + drain.
                        # covers rows {qsize*q ..} and {half + qsize*q ..};
                        # both 32-aligned when qsize == 32.
                        lo = qsize * q
                        if qsize == n_pairs:  # small builds: copy everything
                            nc.scalar.activation(out_sb[:], psum_ts[q][:],
                                                 AF.Copy, scale=SCALE)
                            nc.sync.dma_start(os_[0:n_rows, :],
                                              out_sb[0:n_rows, :])
                        else:
                            nc.scalar.activation(
                                out_sb[lo:lo + qsize, :],
                                psum_ts[q][lo:lo + qsize, :],
                                AF.Copy, scale=SCALE)
                            nc.scalar.activation(
                                out_sb[half + lo:half + lo + qsize, :],
                                psum_ts[q][half + lo:half + lo + qsize, :],
                                AF.Copy, scale=SCALE)
                            nc.sync.dma_start(os_[lo:lo + qsize, :],
                                              out_sb[lo:lo + qsize, :])
                            nc.sync.dma_start(
                                os_[half + lo:half + lo + qsize, :],
                                out_sb[half + lo:half + lo + qsize, :])

    nc.compile()
    return nc


_NC_CACHE = {}


def _get_nc(n_rows=ROWS_PER_CORE):
    if n_rows not in _NC_CACHE:
        _NC_CACHE[n_rows] = build(n_rows)
    return _NC_CACHE[n_rows]


def kernel(x: np.ndarray, y: np.ndarray) -> np.ndarray:
    x = np.ascontiguousarray(np.asarray(x, dtype=np.float32))
    y = np.ascontiguousarray(np.asarray(y, dtype=np.float32))
    assert x.shape == (B, C, H, W) and y.shape == (B, C, H, W)

    nc = _get_nc()
    z = make_ones_const()
    hh = H // 2
    in_maps = []
    for k in range(N_CORES):
        b, h0 = divmod(k, 2)
        h0 *= hh
        in_maps.append({
            "xs": np.ascontiguousarray(x[b, :, h0:h0 + hh, :]),
            "ys": np.ascontiguousarray(y[b, :, h0:h0 + hh, :]),
            "zs": z,
        })
    res = run_bass_kernel_spmd(nc, in_maps, core_ids=list(range(N_CORES)))
    out = np.empty((B, H, W), dtype=np.float32)
    for k in range(N_CORES):
        b, h0 = divmod(k, 2)
        h0 *= hh
        out[b, h0:h0 + hh, :] = res.results[k]["os"]
    return out



# revision 2
# speedup vs baseline: 1.0117x; 1.0117x over previous
"""Trainium2 Bass kernel for nn_Corr — layout-T (no scans).

Math: out = (1/(81*C)) * sum_c [ x*Sy + y*Sx ]
    Sx[w] = sum_{d=0..40} x[w+d]; Sy[w] = sum_{d=1..40} y[w-d]  (zero-padded)

The fp32/scan design is DVE-bound: TensorTensorScanArith runs ~2.7 cyc/elem
serially, a hard ~200us floor. Here the window sums move to the idle
TensorEngine as banded matmuls, which requires W on partitions:

  - Host (free, not timed): cast x,y to bf16 and pre-transpose to
    [tile 16, wm 128, chunk 4, row 8, c 64] per core. DMA is then fully
    contiguous; HBM traffic halves vs fp32.
  - Per 8-row tile: SyT[:,j] = Bb0^T @ yT[:,j] + Bb1^T @ yT[:,j-1] (PSUM,
    fp32), SxT analogous with Bf0/Bf1 against xT. 14 matmuls/tile, banded
    128x128 bf16 stationaries with the 1/(81*C) scale folded in.
  - ScalarE drains Sy/Sx PSUM -> SBUF bf16 (8x [128,512] per tile).
  - Products P1=xT*SyT, P2=yT*SxT elementwise on DVE (bf16 2x) with a few
    on GpSimd for balance.
  - Channel sum (over products s=2 and c=64, free axis): two strided
    TT-add halving levels (2x mode) + one tensor_reduce -> [128, (j,r)].
  - One [128,32] f32 output DMA per tile; host inverse-transposes.
"""
import numpy as np

import concourse.bass as bass
import concourse.tile as tile
from concourse import bacc, mybir
from concourse.bass_utils import run_bass_kernel_spmd

N_CORES = 8
B, C, H, W = 4, 64, 256, 512
MAXD = 40
D = 2 * MAXD + 1  # 81
ROWS_PER_CORE = B * H // N_CORES  # 128
SCALE = 1.0 / (D * C)

RT = 8                       # rows per tile
NT = ROWS_PER_CORE // RT     # 16 tiles
NJ = 4                       # w chunks of 128
NFREE = RT * C               # 512 free elems per chunk

F32 = mybir.dt.float32
BF16 = mybir.dt.bfloat16
AOP = mybir.AluOpType
AF = mybir.ActivationFunctionType
AX = mybir.AxisListType


def make_band_consts() -> np.ndarray:
    """[128, 4*128] bf16: Bb0 | Bb1 | Bf0 | Bf1 (scale folded in).
    B[u', w'] conventions (matmul computes out[w] = sum_u lhsT[u, w] rhs[u, n]):
      Bb0: Sy within chunk    1 <= w-u <= 40
      Bb1: Sy from chunk j-1  88 <= u-w <= 127
      Bf0: Sx within chunk    0 <= u-w <= 40
      Bf1: Sx from chunk j+1  88 <= w-u <= 127
    """
    import ml_dtypes
    u = np.arange(128)[:, None]
    w = np.arange(128)[None, :]
    bb0 = ((w - u >= 1) & (w - u <= 40)).astype(np.float32)
    bb1 = ((u - w >= 88) & (u - w <= 127)).astype(np.float32)
    bf0 = ((u - w >= 0) & (u - w <= 40)).astype(np.float32)
    bf1 = ((w - u >= 88) & (w - u <= 127)).astype(np.float32)
    z = np.concatenate([bb0, bb1, bf0, bf1], axis=1) * SCALE
    return z.astype(ml_dtypes.bfloat16)


def build():
    nc = bacc.Bacc("TRN2", target_bir_lowering=False, debug=False,
                   num_devices=N_CORES)
    xt = nc.dram_tensor("xt", [NT, 128, NJ * NFREE], BF16,
                        kind="ExternalInput").ap()
    yt = nc.dram_tensor("yt", [NT, 128, NJ * NFREE], BF16,
                        kind="ExternalInput").ap()
    bm = nc.dram_tensor("bm", [128, 4 * 128], BF16, kind="ExternalInput").ap()
    os_ = nc.dram_tensor("os", [NT, 128, NJ * RT], F32,
                         kind="ExternalOutput").ap()

    with tile.TileContext(nc) as tc:
        with (
            tc.tile_pool(name="const", bufs=1) as constp,
            tc.tile_pool(name="xt", bufs=4) as xtp,
            tc.tile_pool(name="yt", bufs=4) as ytp,
            tc.tile_pool(name="sy", bufs=3) as syp,
            tc.tile_pool(name="sx", bufs=3) as sxp,
            tc.tile_pool(name="pt", bufs=3) as ptp,
            tc.tile_pool(name="qt", bufs=3) as qtp,
            tc.tile_pool(name="rt", bufs=3) as rtp,
            tc.tile_pool(name="ob", bufs=3) as obp,
            tc.tile_pool(name="ps", bufs=1, space="PSUM") as psp,
        ):
            bsb = constp.tile([128, 4 * 128], BF16)
            nc.sync.dma_start(bsb[:], bm)
            BB0 = bsb[:, 0:128]
            BB1 = bsb[:, 128:256]
            BF0 = bsb[:, 256:384]
            BF1 = bsb[:, 384:512]

            # 2-bank PSUM tiles: banks j,j+1 share a tile so drains can be
            # one [128,1024] ACT op instead of two [128,512]
            sy_ps2 = [psp.tile([128, 2 * NFREE], F32, tag=f"sy{j}",
                               name=f"sy_ps{j}") for j in range(2)]
            sx_ps2 = [psp.tile([128, 2 * NFREE], F32, tag=f"sx{j}",
                               name=f"sx_ps{j}") for j in range(2)]
            sy_ps = [sy_ps2[j // 2][:, (j % 2) * NFREE:(j % 2 + 1) * NFREE]
                     for j in range(NJ)]
            sx_ps = [sx_ps2[j // 2][:, (j % 2) * NFREE:(j % 2 + 1) * NFREE]
                     for j in range(NJ)]

            for t in range(NT):
                xsb = xtp.tile([128, NJ * NFREE], BF16, tag="xt")
                ysb = ytp.tile([128, NJ * NFREE], BF16, tag="yt")
                nc.sync.dma_start(xsb[:], xt[t])
                nc.sync.dma_start(ysb[:], yt[t])

                def ch(ap, j):
                    return ap[:, j * NFREE:(j + 1) * NFREE]

                # ---- banded matmuls (stationary-grouped) ----
                # Sy: Bb0 on chunks 0..3, then Bb1 cross terms 1..3
                for j in range(NJ):
                    nc.tensor.matmul(sy_ps[j][:], BB0, ch(ysb, j),
                                     start=True, stop=(j == 0))
                for j in range(1, NJ):
                    nc.tensor.matmul(sy_ps[j][:], BB1, ch(ysb, j - 1),
                                     start=False, stop=True)
                # Sx: Bf0 on chunks 0..3, then Bf1 cross terms 0..2
                for j in range(NJ):
                    nc.tensor.matmul(sx_ps[j][:], BF0, ch(xsb, j),
                                     start=True, stop=(j == NJ - 1))
                for j in range(NJ - 1):
                    nc.tensor.matmul(sx_ps[j][:], BF1, ch(xsb, j + 1),
                                     start=False, stop=True)

                # ---- PSUM -> SBUF drains (bf16), one op per 2 banks ----
                sysb = syp.tile([128, NJ * NFREE], BF16, tag="sy")
                sxsb = sxp.tile([128, NJ * NFREE], BF16, tag="sx")
                for h in range(2):
                    nc.scalar.activation(
                        sysb[:, h * 2 * NFREE:(h + 1) * 2 * NFREE],
                        sy_ps2[h][:], AF.Copy)
                    nc.scalar.activation(
                        sxsb[:, h * 2 * NFREE:(h + 1) * 2 * NFREE],
                        sx_ps2[h][:], AF.Copy)

                # ---- products: pt halves [P1 (j r c) | P2 (j r c)] ----
                # fully-contiguous 2D slices, 16B-aligned: fastest TT path
                HALF = NJ * RT * C
                pt = ptp.tile([128, 2 * HALF], BF16, tag="pt")
                # P1 = x * Sy (DVE, + P2 j=0); P2 j=1..3 on GpSimd. GpSimd
                # ops issue first: they're slower per-op and gate L1.
                for j in range(1, NJ):
                    nc.gpsimd.tensor_tensor(
                        pt[:, HALF + j * NFREE:HALF + (j + 1) * NFREE],
                        ch(ysb, j), ch(sxsb, j), AOP.mult)
                nc.vector.tensor_tensor(
                    pt[:, HALF:HALF + NFREE],
                    ch(ysb, 0), ch(sxsb, 0), AOP.mult)
                for j in range(NJ):
                    nc.vector.tensor_tensor(
                        pt[:, j * NFREE:(j + 1) * NFREE],
                        ch(xsb, j), ch(sysb, j), AOP.mult)

                # ---- reduction over (s, c): 2 halving adds + reduce ----
                # L1 in two halves so the first can start before GpSimd
                # finishes the j=2,3 products.
                qt = qtp.tile([128, NJ * RT * C], BF16, tag="qt")
                qv = qt[:].rearrange("p (j r c) -> p j r c", j=NJ, r=RT, c=C)
                HH = HALF // 2
                nc.vector.tensor_tensor(
                    qt[:, 0:HH], pt[:, 0:HH], pt[:, HALF:HALF + HH], AOP.add)
                nc.vector.tensor_tensor(
                    qt[:, HH:HALF], pt[:, HH:HALF],
                    pt[:, HALF + HH:2 * HALF], AOP.add)
                # direct grouped reduce over c (1-port op: doesn't contend
                # with GpSimd for the shared SBUF pair)
                ob = obp.tile([128, NJ * RT], F32, tag="ob")
                nc.vector.tensor_reduce(
                    ob[:].rearrange("p (g one) -> p g one", one=1),
                    qt[:].rearrange("p (g c) -> p g c", c=C),
                    AX.X, AOP.add)
                nc.sync.dma_start(os_[t], ob[:])

    nc.compile()
    return nc


_NC_CACHE = {}


def _get_nc():
    if "nc" not in _NC_CACHE:
        _NC_CACHE["nc"] = build()
    return _NC_CACHE["nc"]


def _prep_core(arr_core):
    """[C, 128rows, W] f32 -> [NT, 128wm, NJ*RT*C] bf16 contiguous."""
    import ml_dtypes
    a = arr_core.astype(ml_dtypes.bfloat16)
    a = a.reshape(C, NT, RT, NJ, 128)          # [c, t, r, j, wm]
    a = a.transpose(1, 4, 3, 2, 0)             # [t, wm, j, r, c]
    return np.ascontiguousarray(a.reshape(NT, 128, NJ * RT * C))


def prepare_in_maps(x: np.ndarray, y: np.ndarray):
    bmz = make_band_consts()
    hh = H // 2
    in_maps = []
    for k in range(N_CORES):
        b, h0 = divmod(k, 2)
        h0 *= hh
        in_maps.append({
            "xt": _prep_core(np.ascontiguousarray(x[b, :, h0:h0 + hh, :])),
            "yt": _prep_core(np.ascontiguousarray(y[b, :, h0:h0 + hh, :])),
            "bm": bmz,
        })
    return in_maps


def assemble_out(results):
    hh = H // 2
    out = np.empty((B, H, W), dtype=np.float32)
    for k in range(N_CORES):
        b, h0 = divmod(k, 2)
        h0 *= hh
        o = results[k]["os"]                    # [t, wm, j*RT]
        o = o.reshape(NT, 128, NJ, RT)          # [t, wm, j, r]
        o = o.transpose(0, 3, 2, 1)             # [t, r, j, wm]
        out[b, h0:h0 + hh, :] = o.reshape(hh, W)
    return out


def kernel(x: np.ndarray, y: np.ndarray) -> np.ndarray:
    assert x.shape == (B, C, H, W) and y.shape == (B, C, H, W)
    nc = _get_nc()
    in_maps = prepare_in_maps(x, y)
    res = run_bass_kernel_spmd(nc, in_maps, core_ids=list(range(N_CORES)))
    return assemble_out(res.results)
